# revision 38
# baseline (speedup 1.0000x reference)
"""Trainium2 Bass kernel for nn_BinarizedConv2d.

Math: activation[d, o] = sum_k weight_noise[d, o, k] * x[d, k]
      out[d, o]        = activation[d, o] > bias_noise[d, o]
with D=128 directions, O=256 out channels, K=2304 reduction length.

Sharding: D is split across 8 NeuronCores (16 directions per core) —
embarrassingly parallel, no collectives.

dtype trick: weight_noise and x are exactly 0/1, which fp8e4 represents
exactly; matmul accumulation is always fp32 in PSUM, and popcounts <= 2304
are exact in fp32, so results are bit-identical to the fp32 reference while
moving 4x fewer bytes from HBM (this kernel is HBM-bandwidth-bound on the
9.4 MB/core weight stream).

Per-core kernel: directions are processed as 4 "quads" mapped onto the four
32-column groups of the PE array (tile_position=(0, 32j)), so 4 matvecs run
concurrently. For each K-tile of 128, direction 4q+j's x column is the
stationary operand, broadcast over 32 PE columns with a step-0 AP (M=32);
the pre-transposed weight K-tile [128, 256] is the moving operand (N=256).
The 18 K-tiles accumulate in partitions 32j..32j+31 of PSUM bank q, so each
quad's epilogue is one full-width PSUM->SBUF copy plus one is_gt compare
against a partition-replicated bias on VectorE, and a per-quad uint8 store.

Weights stream as half-quad chunks (1.18 MB, contiguous per partition),
alternating between the two HWDGE rings (SP/ACT); the last half is split in
3 so the PE only trails the stream end by ~3 K-tiles. Every DVE/PE
instruction is structured to need at most ONE semaphore wait (the 64B TPB
instruction structs have a single wait slot): PSUM-copies absorb the PE
wait, compares then see only DVE-produced inputs, and bias replication uses
0-step-AP DMA broadcasts covered by a DVE probe copy.
"""

import numpy as np
import ml_dtypes

D = 128          # directions (ES population)
O = 256          # out channels
K = 2304         # flattened reduction length
T = 18           # K tiles of 128
P = 128          # partitions / K-tile size
NCORES = 8
DPC = D // NCORES  # directions per core

FP8 = ml_dtypes.float8_e4m3

_nc_cache = {}


def _emit(tc, res_ap, wT_ap, xT_ap, bias_ap):
    """Emit the per-core program into TileContext tc."""
    import concourse.mybir as mybir

    nc = tc.nc
    fp8 = mybir.dt.float8e4
    f32 = mybir.dt.float32
    u8 = mybir.dt.uint8

    NQ = DPC // 4  # quads of 4 directions, col-tiled across the PE array
    TH = T // 2    # k-tiles per half (W DMA'd in halves for pipelining)

    with (
        tc.tile_pool(name="w", bufs=1) as wp,
        tc.tile_pool(name="small", bufs=1) as sp,
        tc.tile_pool(name="act", bufs=1) as ap_pool,
        tc.tile_pool(name="ps", bufs=1, space="PSUM") as pp,
    ):
        # x first on the SP ring so no W chunk queues ahead of it (every
        # matmul depends on x).
        x_tile = sp.tile([P, DPC * T], fp8)
        nc.sync.dma_start(out=x_tile[:], in_=xT_ap)
        # bias arrives as 4 DRAM rows (row j = directions 4q+j over quads q),
        # each DMA'd with a 0-step partition AP so partition 32j+r holds
        # direction 4q+j's bias for all r: the per-quad compare is one
        # full-width DVE op.
        bias_rep = sp.tile([P, NQ * O], f32)
        for j in range(4):
            nc.scalar.dma_start(
                out=bias_rep[32 * j : 32 * (j + 1), :],
                in_=bias_ap[j : j + 1, :].broadcast_to((32, NQ * O)),
            )
        # DVE probe depending on the broadcasts: later DVE compares then carry
        # no GpSimd wait (the TPB 64B instruction structs have a single
        # sync-wait slot, and the compares already need a DVE-self wait).
        probe_tile = sp.tile([1, 4], f32)
        nc.vector.tensor_copy(out=probe_tile[:], in_=bias_rep[0:1, 0:4])
        res_tile = sp.tile([P, NQ * O], u8)

        # W arrives as half-quad chunks of [P, TH*4*O] (1.18 MB, contiguous
        # per partition for big SDMA descriptors; th-major so k-tile ranges
        # are contiguous), issued in consume order and alternating between
        # the two HWDGE rings (SP + ACT) so both descriptor queues stream
        # concurrently. The final half is split into 3 pieces so the PE only
        # trails the stream end by ~3 k-tiles.
        NPIECE = 3
        PTH = TH // NPIECE
        HW_ = TH * 4 * O     # elements per half
        PW = PTH * 4 * O     # elements per piece
        w_quads = []
        ring = [nc.sync, nc.scalar]
        issue = 0
        for q in range(NQ):
            halves = []
            for h in range(2):
                if q < NQ - 1 or h == 0:
                    wt = wp.tile([P, HW_], fp8, tag=f"wq{q}h{h}")
                    ring[issue % 2].dma_start(
                        out=wt[:], in_=wT_ap[q][:, h * HW_ : (h + 1) * HW_]
                    )
                    issue += 1
                    halves.append(wt)
                else:
                    pieces = []
                    for pz in range(NPIECE):
                        wt = wp.tile([P, PW], fp8, tag=f"wq{q}h{h}p{pz}")
                        ring[issue % 2].dma_start(
                            out=wt[:],
                            in_=wT_ap[q][:, HW_ + pz * PW : HW_ + (pz + 1) * PW],
                        )
                        issue += 1
                        pieces.append(wt)
                    halves.append(pieces)
            w_quads.append(halves)

        # One PSUM tile spanning all 8 banks; quad q accumulates in bank q's
        # first 256 columns. Direction j of a quad accumulates in partition
        # rows 32j..32j+31 via PE column-group tiling, so the 4 matvecs run
        # concurrently in the array (independent 32-col groups) and the quad
        # epilogue is full-width on DVE. skip_group_check: the per-(q,j)
        # accumulation groups are disjoint (partition x bank), but the group
        # tracker models PSUM flat and can't represent partition-ranged
        # groups; actual has_written accumulate semantics are per element.
        ps_all = pp.tile([P, 8 * 2 * O], f32)
        for q in range(NQ):
            win = slice(q * 2 * O, q * 2 * O + O)
            for t in range(T):
                h, th = divmod(t, TH)
                src = w_quads[q][h]
                if isinstance(src, list):
                    src = src[th // PTH]
                    th = th % PTH
                for j in range(4):
                    d = q * 4 + j
                    # lhsT is x broadcast over 32 columns (step-0 AP): all 32
                    # rows of PE column-group j compute the same matvec, so
                    # the activation fills partitions 32j..32j+31.
                    nc.tensor.matmul(
                        ps_all[32 * j : 32 * (j + 1), win],
                        x_tile[:, d * T + t : d * T + t + 1].broadcast_to((P, 32)),
                        src[:, (th * 4 + j) * O : (th * 4 + j + 1) * O],
                        start=(t == 0),
                        stop=(t == T - 1),
                        tile_position=(0, 32 * j),
                        skip_group_check=True,
                    )
            sl = slice(q * O, (q + 1) * O)
            act_tile = ap_pool.tile([P, O], f32, tag=f"act{q}")
            nc.vector.tensor_copy(out=act_tile[:], in_=ps_all[:, win])
            nc.vector.tensor_tensor(
                out=res_tile[:, sl],
                in0=act_tile[:],
                in1=bias_rep[:, sl],
                op=mybir.AluOpType.is_gt,
            )
            # Per-quad result store: quads 0..2 fly out while later quads
            # still compute; only quad 3's small store is on the tail.
            nc.sync.dma_start(out=res_ap[:, sl], in_=res_tile[0:P:32, sl])


def _build():
    """Build the per-core Bass program (same NEFF on all 8 cores)."""
    import concourse.bacc as bacc
    import concourse.mybir as mybir
    from concourse.tile import TileContext

    # Bacc (not raw Bass): its compile() runs move_matmul_waits_to_ldweights,
    # which splits 2-wait matmuls into LDW-wait + MM-wait (the 64B TPB
    # instruction structs have a single sync-wait slot).
    nc = bacc.Bacc("TRN2", debug=False, enable_asserts=False)

    fp8 = mybir.dt.float8e4
    f32 = mybir.dt.float32
    u8 = mybir.dt.uint8

    # wT[q, p, ((h*9+th)*4 + j)*O + o] = weight_noise[d0+4q+j, o, (h*9+th)*128+p]
    # (pre-transposed host side; one region per quad, h/th-major)
    wT = nc.dram_tensor("wT", [DPC // 4, P, T * 4 * O], fp8, kind="ExternalInput")
    # xT[p, d*T + t] = x[d0+d, t*128+p]
    xT = nc.dram_tensor("xT", [P, DPC * T], fp8, kind="ExternalInput")
    # bias[j, q*O + o] = bias_noise[d0+4q+j, o]
    bias = nc.dram_tensor("bias", [4, (DPC // 4) * O], f32, kind="ExternalInput")
    # res[j, q*O + o] = out[d0+4q+j, o]
    res = nc.dram_tensor("res", [4, (DPC // 4) * O], u8, kind="ExternalOutput")

    with TileContext(nc) as tc:
        _emit(tc, res.ap(), wT.ap(), xT.ap(), bias.ap())
    nc.compile()
    return nc


def prepare_inputs(weight_noise, bias_noise, x):
    """Host-side dtype cast + layout transform + sharding. Exact (0/1 -> fp8)."""
    w8 = np.asarray(weight_noise).astype(FP8)           # [D, O, K]
    # wT[d, p, t, o] = w[d, o, t*128+p]
    wT = np.ascontiguousarray(
        w8.reshape(D, O, T, P).transpose(0, 3, 2, 1)
    ).reshape(D, P, T * O)
    x8 = np.asarray(x).astype(FP8)                      # [D, K]
    xTfull = np.ascontiguousarray(x8.reshape(D, T, P).transpose(2, 0, 1))  # [P, D, T]
    b32 = np.asarray(bias_noise).astype(np.float32)

    in_maps = []
    for c in range(NCORES):
        sl = slice(c * DPC, (c + 1) * DPC)
        # [d, p, t, o] -> [q, p, t, j, o] -> one region per quad (t-major)
        wc = (
            wT[sl]
            .reshape(DPC // 4, 4, P, T, O)
            .transpose(0, 2, 3, 1, 4)
            .reshape(DPC // 4, P, T * 4 * O)
        )
        # bias[j, q*O+o] = bias_noise[d0 + 4q + j, o]
        bc = (
            b32[sl]
            .reshape(DPC // 4, 4, O)
            .transpose(1, 0, 2)
            .reshape(4, (DPC // 4) * O)
        )
        in_maps.append(
            {
                "wT": np.ascontiguousarray(wc),
                "xT": np.ascontiguousarray(xTfull[:, sl, :]).reshape(P, DPC * T),
                "bias": np.ascontiguousarray(bc),
            }
        )
    return in_maps


def run(weight_noise, bias_noise, x, trace=False, **spmd_kwargs):
    """Run on the 8 NeuronCores; returns (bool [D, O] output, BassKernelResults)."""
    from concourse.bass_utils import run_bass_kernel_spmd

    in_maps = prepare_inputs(weight_noise, bias_noise, x)
    if "nc" in _nc_cache:
        nc = _nc_cache["nc"]
    else:
        nc = _nc_cache["nc"] = _build()
    r = run_bass_kernel_spmd(
        nc, in_maps, core_ids=list(range(NCORES)), trace=trace, **spmd_kwargs
    )
    out = np.concatenate(
        [
            r.results[c]["res"]
            .reshape(4, DPC // 4, O)
            .transpose(1, 0, 2)
            .reshape(DPC, O)
            for c in range(NCORES)
        ],
        axis=0,
    )
    return out.astype(bool), r


def kernel(weight_noise, bias_noise, x):
    out, _ = run(weight_noise, bias_noise, x)
    return out


# revision 40
# speedup vs baseline: 1.0618x; 1.0618x over previous
"""Trainium2 Bass kernel for nn_BinarizedConv2d.

Math: activation[d, o] = sum_k weight_noise[d, o, k] * x[d, k]
      out[d, o]        = activation[d, o] > bias_noise[d, o]
with D=128 directions, O=256 out channels, K=2304 reduction length.

Sharding: D is split across 8 NeuronCores (16 directions per core) —
embarrassingly parallel, no collectives.

dtype trick: weight_noise and x are exactly 0/1, which fp8e4 represents
exactly; matmul accumulation is always fp32 in PSUM, and popcounts <= 2304
are exact in fp32, so results are bit-identical to the fp32 reference while
moving 4x fewer bytes from HBM (this kernel is HBM-bandwidth-bound on the
9.4 MB/core weight stream).

Per-core kernel: directions are processed as 4 "quads" mapped onto the four
32-column groups of the PE array (tile_position=(0, 32j)), so 4 matvecs run
concurrently. For each K-tile of 128, direction 4q+j's x column is the
stationary operand, broadcast over 32 PE columns with a step-0 AP (M=32);
the pre-transposed weight K-tile [128, 256] is the moving operand (N=256).
The 18 K-tiles accumulate in partitions 32j..32j+31 of PSUM bank q, so each
quad's epilogue is one full-width PSUM->SBUF copy plus one is_gt compare
against a partition-replicated bias on VectorE, and a per-quad uint8 store.

Weights stream as half-quad chunks (1.18 MB, contiguous per partition),
alternating between the two HWDGE rings (SP/ACT); the last half is split in
3 so the PE only trails the stream end by ~3 K-tiles. Every DVE/PE
instruction is structured to need at most ONE semaphore wait (the 64B TPB
instruction structs have a single wait slot): PSUM-copies absorb the PE
wait, compares then see only DVE-produced inputs, and bias replication uses
0-step-AP DMA broadcasts covered by a DVE probe copy.
"""

import numpy as np
import ml_dtypes

D = 128          # directions (ES population)
O = 256          # out channels
K = 2304         # flattened reduction length
T = 18           # K tiles of 128
P = 128          # partitions / K-tile size
NCORES = 8
DPC = D // NCORES  # directions per core

FP8 = ml_dtypes.float8_e4m3

_nc_cache = {}


def _emit(tc, res_ap, wT_ap, xT_ap, bias_ap):
    """Emit the per-core program into TileContext tc."""
    import concourse.mybir as mybir

    nc = tc.nc
    fp8 = mybir.dt.float8e4
    f32 = mybir.dt.float32
    u8 = mybir.dt.uint8

    NQ = DPC // 4  # quads of 4 directions, col-tiled across the PE array
    TH = T // 2    # k-tiles per half (W DMA'd in halves for pipelining)

    with (
        tc.tile_pool(name="w", bufs=1) as wp,
        tc.tile_pool(name="small", bufs=1) as sp,
        tc.tile_pool(name="act", bufs=1) as ap_pool,
        tc.tile_pool(name="ps", bufs=1, space="PSUM") as pp,
    ):
        # x first on the SP ring so no W chunk queues ahead of it (every
        # matmul depends on x).
        x_tile = sp.tile([P, DPC * T], fp8)
        nc.sync.dma_start(out=x_tile[:], in_=xT_ap)
        # bias arrives as 4 DRAM rows (row j = directions 4q+j over quads q),
        # each DMA'd with a 0-step partition AP so partition 32j+r holds
        # direction 4q+j's bias for all r: the per-quad compare is one
        # full-width DVE op.
        bias_rep = sp.tile([P, NQ * O], f32)
        for j in range(4):
            nc.scalar.dma_start(
                out=bias_rep[32 * j : 32 * (j + 1), :],
                in_=bias_ap[j : j + 1, :].broadcast_to((32, NQ * O)),
            )
        # DVE probe depending on the broadcasts: later DVE compares then carry
        # no GpSimd wait (the TPB 64B instruction structs have a single
        # sync-wait slot, and the compares already need a DVE-self wait).
        probe_tile = sp.tile([1, 4], f32)
        nc.vector.tensor_copy(out=probe_tile[:], in_=bias_rep[0:1, 0:4])

        # W arrives as half-quad chunks of [P, TH*4*O] (1.18 MB, contiguous
        # per partition for big SDMA descriptors; th-major so k-tile ranges
        # are contiguous), issued in consume order and alternating between
        # the two HWDGE rings (SP + ACT) so both descriptor queues stream
        # concurrently. The final half is split into 3 pieces so the PE only
        # trails the stream end by ~3 k-tiles.
        NPIECE = 3
        PTH = TH // NPIECE
        HW_ = TH * 4 * O     # elements per half
        PW = PTH * 4 * O     # elements per piece
        w_quads = []
        ring = [nc.sync, nc.scalar]
        issue = 0
        for q in range(NQ):
            halves = []
            for h in range(2):
                if q < NQ - 1 or h == 0:
                    wt = wp.tile([P, HW_], fp8, tag=f"wq{q}h{h}")
                    ring[issue % 2].dma_start(
                        out=wt[:], in_=wT_ap[q][:, h * HW_ : (h + 1) * HW_]
                    )
                    issue += 1
                    halves.append(wt)
                else:
                    pieces = []
                    for pz in range(NPIECE):
                        wt = wp.tile([P, PW], fp8, tag=f"wq{q}h{h}p{pz}")
                        ring[issue % 2].dma_start(
                            out=wt[:],
                            in_=wT_ap[q][:, HW_ + pz * PW : HW_ + (pz + 1) * PW],
                        )
                        issue += 1
                        pieces.append(wt)
                    halves.append(pieces)
            w_quads.append(halves)

        # One PSUM tile spanning all 8 banks; quad q accumulates in bank q's
        # first 256 columns. Direction j of a quad accumulates in partition
        # rows 32j..32j+31 via PE column-group tiling, so the 4 matvecs run
        # concurrently in the array (independent 32-col groups) and the quad
        # epilogue is full-width on DVE. skip_group_check: the per-(q,j)
        # accumulation groups are disjoint (partition x bank), but the group
        # tracker models PSUM flat and can't represent partition-ranged
        # groups; actual has_written accumulate semantics are per element.
        ps_all = pp.tile([P, 8 * 2 * O], f32)
        for q in range(NQ):
            win = slice(q * 2 * O, q * 2 * O + O)
            for t in range(T):
                h, th = divmod(t, TH)
                src = w_quads[q][h]
                if isinstance(src, list):
                    src = src[th // PTH]
                    th = th % PTH
                for j in range(4):
                    d = q * 4 + j
                    # lhsT is x broadcast over 32 columns (step-0 AP): all 32
                    # rows of PE column-group j compute the same matvec, so
                    # the activation fills partitions 32j..32j+31.
                    nc.tensor.matmul(
                        ps_all[32 * j : 32 * (j + 1), win],
                        x_tile[:, d * T + t : d * T + t + 1].broadcast_to((P, 32)),
                        src[:, (th * 4 + j) * O : (th * 4 + j + 1) * O],
                        start=(t == 0),
                        stop=(t == T - 1),
                        tile_position=(0, 32 * j),
                        skip_group_check=True,
                    )
            sl = slice(q * O, (q + 1) * O)
            # Fused epilogue: res = (ps + 0.0) is_gt bias, one DVE op per
            # quad, reading PSUM directly. Per-quad res tiles: no WAW between
            # quads, so each op's only semaphore wait is the PE one.
            res_q = ap_pool.tile([P, O], u8, tag=f"res{q}")
            nc.vector.scalar_tensor_tensor(
                out=res_q[:],
                in0=ps_all[:, win],
                scalar=0.0,
                in1=bias_rep[:, sl],
                op0=mybir.AluOpType.add,
                op1=mybir.AluOpType.is_gt,
            )
            # Per-quad result store: quads 0..2 fly out while later quads
            # still compute; only quad 3's small store is on the tail.
            nc.sync.dma_start(out=res_ap[:, sl], in_=res_q[0:P:32, :])


def _build():
    """Build the per-core Bass program (same NEFF on all 8 cores)."""
    import concourse.bacc as bacc
    import concourse.mybir as mybir
    from concourse.tile import TileContext

    # Bacc (not raw Bass): its compile() runs move_matmul_waits_to_ldweights,
    # which splits 2-wait matmuls into LDW-wait + MM-wait (the 64B TPB
    # instruction structs have a single sync-wait slot).
    nc = bacc.Bacc("TRN2", debug=False, enable_asserts=False)

    fp8 = mybir.dt.float8e4
    f32 = mybir.dt.float32
    u8 = mybir.dt.uint8

    # wT[q, p, ((h*9+th)*4 + j)*O + o] = weight_noise[d0+4q+j, o, (h*9+th)*128+p]
    # (pre-transposed host side; one region per quad, h/th-major)
    wT = nc.dram_tensor("wT", [DPC // 4, P, T * 4 * O], fp8, kind="ExternalInput")
    # xT[p, d*T + t] = x[d0+d, t*128+p]
    xT = nc.dram_tensor("xT", [P, DPC * T], fp8, kind="ExternalInput")
    # bias[j, q*O + o] = bias_noise[d0+4q+j, o]
    bias = nc.dram_tensor("bias", [4, (DPC // 4) * O], f32, kind="ExternalInput")
    # res[j, q*O + o] = out[d0+4q+j, o]
    res = nc.dram_tensor("res", [4, (DPC // 4) * O], u8, kind="ExternalOutput")

    with TileContext(nc) as tc:
        _emit(tc, res.ap(), wT.ap(), xT.ap(), bias.ap())
    nc.compile()
    return nc


def prepare_inputs(weight_noise, bias_noise, x):
    """Host-side dtype cast + layout transform + sharding. Exact (0/1 -> fp8)."""
    w8 = np.asarray(weight_noise).astype(FP8)           # [D, O, K]
    # wT[d, p, t, o] = w[d, o, t*128+p]
    wT = np.ascontiguousarray(
        w8.reshape(D, O, T, P).transpose(0, 3, 2, 1)
    ).reshape(D, P, T * O)
    x8 = np.asarray(x).astype(FP8)                      # [D, K]
    xTfull = np.ascontiguousarray(x8.reshape(D, T, P).transpose(2, 0, 1))  # [P, D, T]
    b32 = np.asarray(bias_noise).astype(np.float32)

    in_maps = []
    for c in range(NCORES):
        sl = slice(c * DPC, (c + 1) * DPC)
        # [d, p, t, o] -> [q, p, t, j, o] -> one region per quad (t-major)
        wc = (
            wT[sl]
            .reshape(DPC // 4, 4, P, T, O)
            .transpose(0, 2, 3, 1, 4)
            .reshape(DPC // 4, P, T * 4 * O)
        )
        # bias[j, q*O+o] = bias_noise[d0 + 4q + j, o]
        bc = (
            b32[sl]
            .reshape(DPC // 4, 4, O)
            .transpose(1, 0, 2)
            .reshape(4, (DPC // 4) * O)
        )
        in_maps.append(
            {
                "wT": np.ascontiguousarray(wc),
                "xT": np.ascontiguousarray(xTfull[:, sl, :]).reshape(P, DPC * T),
                "bias": np.ascontiguousarray(bc),
            }
        )
    return in_maps


def run(weight_noise, bias_noise, x, trace=False, **spmd_kwargs):
    """Run on the 8 NeuronCores; returns (bool [D, O] output, BassKernelResults)."""
    from concourse.bass_utils import run_bass_kernel_spmd

    in_maps = prepare_inputs(weight_noise, bias_noise, x)
    if "nc" in _nc_cache:
        nc = _nc_cache["nc"]
    else:
        nc = _nc_cache["nc"] = _build()
    r = run_bass_kernel_spmd(
        nc, in_maps, core_ids=list(range(NCORES)), trace=trace, **spmd_kwargs
    )
    out = np.concatenate(
        [
            r.results[c]["res"]
            .reshape(4, DPC // 4, O)
            .transpose(1, 0, 2)
            .reshape(DPC, O)
            for c in range(NCORES)
        ],
        axis=0,
    )
    return out.astype(bool), r


def kernel(weight_noise, bias_noise, x):
    out, _ = run(weight_noise, bias_noise, x)
    return out


# revision 43
# speedup vs baseline: 1.1339x; 1.0679x over previous
"""Trainium2 Bass kernel for nn_BinarizedConv2d.

Math: activation[d, o] = sum_k weight_noise[d, o, k] * x[d, k]
      out[d, o]        = activation[d, o] > bias_noise[d, o]
with D=128 directions, O=256 out channels, K=2304 reduction length.

Sharding: D is split across 8 NeuronCores (16 directions per core) —
embarrassingly parallel, no collectives.

dtype trick: weight_noise and x are exactly 0/1, which fp8e4 represents
exactly; matmul accumulation is always fp32 in PSUM, and popcounts <= 2304
are exact in fp32, so results are bit-identical to the fp32 reference while
moving 4x fewer bytes from HBM (this kernel is HBM-bandwidth-bound on the
9.4 MB/core weight stream).

Per-core kernel: directions are processed as 4 "quads" mapped onto the four
32-column groups of the PE array (tile_position=(0, 32j)), so 4 matvecs run
concurrently. For each K-tile of 128, direction 4q+j's x column is the
stationary operand, broadcast over 32 PE columns with a step-0 AP (M=32);
the pre-transposed weight K-tile [128, 256] is the moving operand (N=256).
The 18 K-tiles accumulate in partitions 32j..32j+31 of PSUM bank q, so each
quad's epilogue is a single fused VectorE op ((psum + 0) is_gt bias) against
a partition-replicated bias, plus a per-quad uint8 store.

Weights stream as half-quad chunks (1.18 MB, contiguous per partition),
alternating between the two HWDGE rings (SP/ACT); the last half is split in
3 so the PE only trails the stream end by ~3 K-tiles. Every DVE/PE
instruction is structured to need at most ONE semaphore wait (the 64B TPB
instruction structs have a single wait slot): per-quad result tiles avoid
write-after-write waits, and the bias replication (0-step-AP DMA broadcast)
is covered by a DVE probe copy so compares only wait on the PE.
"""

import numpy as np
import ml_dtypes

D = 128          # directions (ES population)
O = 256          # out channels
K = 2304         # flattened reduction length
T = 18           # K tiles of 128
P = 128          # partitions / K-tile size
NCORES = 8
DPC = D // NCORES  # directions per core

FP8 = ml_dtypes.float8_e4m3

_nc_cache = {}


def _emit(tc, res_ap, wT_ap, xT_ap, bias_ap):
    """Emit the per-core program into TileContext tc."""
    import concourse.mybir as mybir

    nc = tc.nc
    fp8 = mybir.dt.float8e4
    f32 = mybir.dt.float32
    u8 = mybir.dt.uint8

    NQ = DPC // 4  # quads of 4 directions, col-tiled across the PE array
    TH = T // 2    # k-tiles per half (W DMA'd in halves for pipelining)

    with (
        tc.tile_pool(name="w", bufs=1) as wp,
        tc.tile_pool(name="small", bufs=1) as sp,
        tc.tile_pool(name="act", bufs=1) as ap_pool,
        tc.tile_pool(name="ps", bufs=1, space="PSUM") as pp,
    ):
        # x first on the SP ring so no W chunk queues ahead of it (every
        # matmul depends on x).
        x_tile = sp.tile([P, DPC * T], fp8)
        nc.sync.dma_start(out=x_tile[:], in_=xT_ap)
        # bias arrives as 4 DRAM rows (row j = directions 4q+j over quads q),
        # each DMA'd with a 0-step partition AP so partition 32j+r holds
        # direction 4q+j's bias for all r: the per-quad compare is one
        # full-width DVE op.
        bias_rep = sp.tile([P, NQ * O], f32)
        for j in range(4):
            nc.scalar.dma_start(
                out=bias_rep[32 * j : 32 * (j + 1), :],
                in_=bias_ap[j : j + 1, :].broadcast_to((32, NQ * O)),
            )
        # DVE probe depending on the broadcasts: later DVE compares then carry
        # no GpSimd wait (the TPB 64B instruction structs have a single
        # sync-wait slot, and the compares already need a DVE-self wait).
        probe_tile = sp.tile([1, 4], f32)
        nc.vector.tensor_copy(out=probe_tile[:], in_=bias_rep[0:1, 0:4])

        # W arrives as half-quad chunks of [P, TH*4*O] (1.18 MB, contiguous
        # per partition for big SDMA descriptors; th-major so k-tile ranges
        # are contiguous), issued in consume order and alternating between
        # the two HWDGE rings (SP + ACT) so both descriptor queues stream
        # concurrently. The final half is split into 3 pieces so the PE only
        # trails the stream end by ~3 k-tiles.
        NPIECE = 3
        PTH = TH // NPIECE
        HW_ = TH * 4 * O     # elements per half
        PW = PTH * 4 * O     # elements per piece
        w_quads = []
        ring = [nc.sync, nc.scalar]
        issue = 0
        for q in range(NQ):
            halves = []
            for h in range(2):
                if q < NQ - 1 or h == 0:
                    wt = wp.tile([P, HW_], fp8, tag=f"wq{q}h{h}")
                    ring[issue % 2].dma_start(
                        out=wt[:], in_=wT_ap[q][:, h * HW_ : (h + 1) * HW_]
                    )
                    issue += 1
                    halves.append(wt)
                else:
                    pieces = []
                    for pz in range(NPIECE):
                        wt = wp.tile([P, PW], fp8, tag=f"wq{q}h{h}p{pz}")
                        ring[issue % 2].dma_start(
                            out=wt[:],
                            in_=wT_ap[q][:, HW_ + pz * PW : HW_ + (pz + 1) * PW],
                        )
                        issue += 1
                        pieces.append(wt)
                    halves.append(pieces)
            w_quads.append(halves)

        # One PSUM tile spanning all 8 banks; quad q accumulates in bank q's
        # first 256 columns. Direction j of a quad accumulates in partition
        # rows 32j..32j+31 via PE column-group tiling, so the 4 matvecs run
        # concurrently in the array (independent 32-col groups) and the quad
        # epilogue is full-width on DVE. skip_group_check: the per-(q,j)
        # accumulation groups are disjoint (partition x bank), but the group
        # tracker models PSUM flat and can't represent partition-ranged
        # groups; actual has_written accumulate semantics are per element.
        ps_all = pp.tile([P, 8 * 2 * O], f32)
        for q in range(NQ):
            win = slice(q * 2 * O, q * 2 * O + O)
            for t in range(T):
                h, th = divmod(t, TH)
                src = w_quads[q][h]
                if isinstance(src, list):
                    src = src[th // PTH]
                    th = th % PTH
                for j in range(4):
                    d = q * 4 + j
                    # lhsT is x broadcast over 32 columns (step-0 AP): all 32
                    # rows of PE column-group j compute the same matvec, so
                    # the activation fills partitions 32j..32j+31.
                    nc.tensor.matmul(
                        ps_all[32 * j : 32 * (j + 1), win],
                        x_tile[:, d * T + t : d * T + t + 1].broadcast_to((P, 32)),
                        src[:, (th * 4 + j) * O : (th * 4 + j + 1) * O],
                        start=(t == 0),
                        stop=(t == T - 1),
                        tile_position=(0, 32 * j),
                        skip_group_check=True,
                    )
            sl = slice(q * O, (q + 1) * O)
            # Fused epilogue: res = (ps + 0.0) is_gt bias, one DVE op per
            # quad, reading PSUM directly. Per-quad res tiles: no WAW between
            # quads, so each op's only semaphore wait is the PE one.
            res_q = ap_pool.tile([P, O], u8, tag=f"res{q}")
            nc.vector.scalar_tensor_tensor(
                out=res_q[:],
                in0=ps_all[:, win],
                scalar=0.0,
                in1=bias_rep[:, sl],
                op0=mybir.AluOpType.add,
                op1=mybir.AluOpType.is_gt,
            )
            # Per-quad result store: quads 0..2 fly out while later quads
            # still compute; only quad 3's small store is on the tail.
            nc.sync.dma_start(out=res_ap[:, sl], in_=res_q[0:P:32, :])


def _build():
    """Build the per-core Bass program (same NEFF on all 8 cores)."""
    import concourse.bacc as bacc
    import concourse.mybir as mybir
    from concourse.tile import TileContext

    # Bacc (not raw Bass): its compile() runs move_matmul_waits_to_ldweights,
    # which splits 2-wait matmuls into LDW-wait + MM-wait (the 64B TPB
    # instruction structs have a single sync-wait slot).
    nc = bacc.Bacc("TRN2", debug=False, enable_asserts=False)

    fp8 = mybir.dt.float8e4
    f32 = mybir.dt.float32
    u8 = mybir.dt.uint8

    # wT[q, p, ((h*9+th)*4 + j)*O + o] = weight_noise[d0+4q+j, o, (h*9+th)*128+p]
    # (pre-transposed host side; one region per quad, h/th-major)
    wT = nc.dram_tensor("wT", [DPC // 4, P, T * 4 * O], fp8, kind="ExternalInput")
    # xT[p, d*T + t] = x[d0+d, t*128+p]
    xT = nc.dram_tensor("xT", [P, DPC * T], fp8, kind="ExternalInput")
    # bias[j, q*O + o] = bias_noise[d0+4q+j, o]
    bias = nc.dram_tensor("bias", [4, (DPC // 4) * O], f32, kind="ExternalInput")
    # res[j, q*O + o] = out[d0+4q+j, o]
    res = nc.dram_tensor("res", [4, (DPC // 4) * O], u8, kind="ExternalOutput")

    with TileContext(nc) as tc:
        _emit(tc, res.ap(), wT.ap(), xT.ap(), bias.ap())
    nc.compile()
    return nc


def prepare_inputs(weight_noise, bias_noise, x):
    """Host-side dtype cast + layout transform + sharding. Exact (0/1 -> fp8)."""
    w8 = np.asarray(weight_noise).astype(FP8)           # [D, O, K]
    # wT[d, p, t, o] = w[d, o, t*128+p]
    wT = np.ascontiguousarray(
        w8.reshape(D, O, T, P).transpose(0, 3, 2, 1)
    ).reshape(D, P, T * O)
    x8 = np.asarray(x).astype(FP8)                      # [D, K]
    xTfull = np.ascontiguousarray(x8.reshape(D, T, P).transpose(2, 0, 1))  # [P, D, T]
    b32 = np.asarray(bias_noise).astype(np.float32)

    in_maps = []
    for c in range(NCORES):
        sl = slice(c * DPC, (c + 1) * DPC)
        # [d, p, t, o] -> [q, p, t, j, o] -> one region per quad (t-major)
        wc = (
            wT[sl]
            .reshape(DPC // 4, 4, P, T, O)
            .transpose(0, 2, 3, 1, 4)
            .reshape(DPC // 4, P, T * 4 * O)
        )
        # bias[j, q*O+o] = bias_noise[d0 + 4q + j, o]
        bc = (
            b32[sl]
            .reshape(DPC // 4, 4, O)
            .transpose(1, 0, 2)
            .reshape(4, (DPC // 4) * O)
        )
        in_maps.append(
            {
                "wT": np.ascontiguousarray(wc),
                "xT": np.ascontiguousarray(xTfull[:, sl, :]).reshape(P, DPC * T),
                "bias": np.ascontiguousarray(bc),
            }
        )
    return in_maps


def run(weight_noise, bias_noise, x, trace=False, **spmd_kwargs):
    """Run on the 8 NeuronCores; returns (bool [D, O] output, BassKernelResults)."""
    from concourse.bass_utils import run_bass_kernel_spmd

    in_maps = prepare_inputs(weight_noise, bias_noise, x)
    if "nc" in _nc_cache:
        nc = _nc_cache["nc"]
    else:
        nc = _nc_cache["nc"] = _build()
    r = run_bass_kernel_spmd(
        nc, in_maps, core_ids=list(range(NCORES)), trace=trace, **spmd_kwargs
    )
    out = np.concatenate(
        [
            r.results[c]["res"]
            .reshape(4, DPC // 4, O)
            .transpose(1, 0, 2)
            .reshape(DPC, O)
            for c in range(NCORES)
        ],
        axis=0,
    )
    return out.astype(bool), r


def kernel(weight_noise, bias_noise, x):
    out, _ = run(weight_noise, bias_noise, x)
    return out


# revision 44
# speedup vs baseline: 1.1418x; 1.0069x over previous
"""Trainium2 Bass kernel for nn_BinarizedConv2d.

Math: activation[d, o] = sum_k weight_noise[d, o, k] * x[d, k]
      out[d, o]        = activation[d, o] > bias_noise[d, o]
with D=128 directions, O=256 out channels, K=2304 reduction length.

Sharding: D is split across 8 NeuronCores (16 directions per core) —
embarrassingly parallel, no collectives.

dtype trick: weight_noise and x are exactly 0/1, which fp8e4 represents
exactly; matmul accumulation is always fp32 in PSUM, and popcounts <= 2304
are exact in fp32, so results are bit-identical to the fp32 reference while
moving 4x fewer bytes from HBM (this kernel is HBM-bandwidth-bound on the
9.4 MB/core weight stream).

Per-core kernel: directions are processed as 4 "quads" mapped onto the four
32-column groups of the PE array (tile_position=(0, 32j)), so 4 matvecs run
concurrently. For each K-tile of 128, direction 4q+j's x column is the
stationary operand, broadcast over 32 PE columns with a step-0 AP (M=32);
the pre-transposed weight K-tile [128, 256] is the moving operand (N=256).
The 18 K-tiles accumulate in partitions 32j..32j+31 of PSUM bank q, so each
quad's epilogue is a single fused VectorE op ((psum + 0) is_gt bias) against
a partition-replicated bias, plus a per-quad uint8 store.

Weights stream as half-quad chunks (1.18 MB, contiguous per partition),
alternating between the two HWDGE rings (SP/ACT); the last half is split in
3 so the PE only trails the stream end by ~3 K-tiles. Every DVE/PE
instruction is structured to need at most ONE semaphore wait (the 64B TPB
instruction structs have a single wait slot): per-quad result tiles avoid
write-after-write waits, and the bias replication (0-step-AP DMA broadcast)
is covered by a DVE probe copy so compares only wait on the PE.
"""

import numpy as np
import ml_dtypes

D = 128          # directions (ES population)
O = 256          # out channels
K = 2304         # flattened reduction length
T = 18           # K tiles of 128
P = 128          # partitions / K-tile size
NCORES = 8
DPC = D // NCORES  # directions per core

FP8 = ml_dtypes.float8_e4m3

_nc_cache = {}


def _emit(tc, res_ap, wT_ap, xT_ap, bias_ap):
    """Emit the per-core program into TileContext tc."""
    import concourse.mybir as mybir

    nc = tc.nc
    fp8 = mybir.dt.float8e4
    f32 = mybir.dt.float32
    u8 = mybir.dt.uint8

    NQ = DPC // 4  # quads of 4 directions, col-tiled across the PE array
    TH = T // 2    # k-tiles per half (W DMA'd in halves for pipelining)

    with (
        tc.tile_pool(name="w", bufs=1) as wp,
        tc.tile_pool(name="small", bufs=1) as sp,
        tc.tile_pool(name="act", bufs=1) as ap_pool,
        tc.tile_pool(name="ps", bufs=1, space="PSUM") as pp,
    ):
        # x first on the SP ring so no W chunk queues ahead of it (every
        # matmul depends on x).
        x_tile = sp.tile([P, DPC * T], fp8)
        nc.sync.dma_start(out=x_tile[:], in_=xT_ap)
        # W arrives as half-quad chunks of [P, TH*4*O] (1.18 MB, contiguous
        # per partition for big SDMA descriptors; th-major so k-tile ranges
        # are contiguous), issued in consume order and alternating between
        # the two HWDGE rings (SP + ACT) so both descriptor queues stream
        # concurrently. The final half is split into 3 pieces so the PE only
        # trails the stream end by ~3 k-tiles.
        NPIECE = 3
        PTH = TH // NPIECE
        HW_ = TH * 4 * O     # elements per half
        PW = PTH * 4 * O     # elements per piece
        w_quads = []
        ring = [nc.sync, nc.scalar]
        issue = 0
        for q in range(NQ):
            halves = []
            for h in range(2):
                if q < NQ - 1 or h == 0:
                    wt = wp.tile([P, HW_], fp8, tag=f"wq{q}h{h}")
                    ring[issue % 2].dma_start(
                        out=wt[:], in_=wT_ap[q][:, h * HW_ : (h + 1) * HW_]
                    )
                    issue += 1
                    halves.append(wt)
                else:
                    pieces = []
                    for pz in range(NPIECE):
                        wt = wp.tile([P, PW], fp8, tag=f"wq{q}h{h}p{pz}")
                        ring[issue % 2].dma_start(
                            out=wt[:],
                            in_=wT_ap[q][:, HW_ + pz * PW : HW_ + (pz + 1) * PW],
                        )
                        issue += 1
                        pieces.append(wt)
                    halves.append(pieces)
            w_quads.append(halves)

        # bias arrives as 4 DRAM rows (row j = directions 4q+j over quads q),
        # each DMA'd with a 0-step partition AP so partition 32j+r holds
        # direction 4q+j's bias for all r. Issued after the W chunks so the
        # stream-critical weight data is not queued behind the 128 KB of
        # replicated bias writes; ring FIFO still lands bias well before the
        # first quad's epilogue. A DVE probe copy then absorbs the DMA wait
        # so the fused compares only ever wait on the PE semaphore (the TPB
        # 64B instruction structs have a single sync-wait slot).
        bias_rep = sp.tile([P, NQ * O], f32)
        for j in range(4):
            nc.scalar.dma_start(
                out=bias_rep[32 * j : 32 * (j + 1), :],
                in_=bias_ap[j : j + 1, :].broadcast_to((32, NQ * O)),
            )
        probe_tile = sp.tile([1, 4], f32)
        nc.vector.tensor_copy(out=probe_tile[:], in_=bias_rep[0:1, 0:4])

        # One PSUM tile spanning all 8 banks; quad q accumulates in bank q's
        # first 256 columns. Direction j of a quad accumulates in partition
        # rows 32j..32j+31 via PE column-group tiling, so the 4 matvecs run
        # concurrently in the array (independent 32-col groups) and the quad
        # epilogue is full-width on DVE. skip_group_check: the per-(q,j)
        # accumulation groups are disjoint (partition x bank), but the group
        # tracker models PSUM flat and can't represent partition-ranged
        # groups; actual has_written accumulate semantics are per element.
        ps_all = pp.tile([P, 8 * 2 * O], f32)
        for q in range(NQ):
            win = slice(q * 2 * O, q * 2 * O + O)
            for t in range(T):
                h, th = divmod(t, TH)
                src = w_quads[q][h]
                if isinstance(src, list):
                    src = src[th // PTH]
                    th = th % PTH
                for j in range(4):
                    d = q * 4 + j
                    # lhsT is x broadcast over 32 columns (step-0 AP): all 32
                    # rows of PE column-group j compute the same matvec, so
                    # the activation fills partitions 32j..32j+31.
                    nc.tensor.matmul(
                        ps_all[32 * j : 32 * (j + 1), win],
                        x_tile[:, d * T + t : d * T + t + 1].broadcast_to((P, 32)),
                        src[:, (th * 4 + j) * O : (th * 4 + j + 1) * O],
                        start=(t == 0),
                        stop=(t == T - 1),
                        tile_position=(0, 32 * j),
                        skip_group_check=True,
                    )
            sl = slice(q * O, (q + 1) * O)
            # Fused epilogue: res = (ps + 0.0) is_gt bias, one DVE op per
            # quad, reading PSUM directly. Per-quad res tiles: no WAW between
            # quads, so each op's only semaphore wait is the PE one.
            res_q = ap_pool.tile([P, O], u8, tag=f"res{q}")
            nc.vector.scalar_tensor_tensor(
                out=res_q[:],
                in0=ps_all[:, win],
                scalar=0.0,
                in1=bias_rep[:, sl],
                op0=mybir.AluOpType.add,
                op1=mybir.AluOpType.is_gt,
            )
            # Per-quad result store: quads 0..2 fly out while later quads
            # still compute; only quad 3's small store is on the tail.
            nc.scalar.dma_start(out=res_ap[:, sl], in_=res_q[0:P:32, :])


def _build():
    """Build the per-core Bass program (same NEFF on all 8 cores)."""
    import concourse.bacc as bacc
    import concourse.mybir as mybir
    from concourse.tile import TileContext

    # Bacc (not raw Bass): its compile() runs move_matmul_waits_to_ldweights,
    # which splits 2-wait matmuls into LDW-wait + MM-wait (the 64B TPB
    # instruction structs have a single sync-wait slot).
    nc = bacc.Bacc("TRN2", debug=False, enable_asserts=False)

    fp8 = mybir.dt.float8e4
    f32 = mybir.dt.float32
    u8 = mybir.dt.uint8

    # wT[q, p, ((h*9+th)*4 + j)*O + o] = weight_noise[d0+4q+j, o, (h*9+th)*128+p]
    # (pre-transposed host side; one region per quad, h/th-major)
    wT = nc.dram_tensor("wT", [DPC // 4, P, T * 4 * O], fp8, kind="ExternalInput")
    # xT[p, d*T + t] = x[d0+d, t*128+p]
    xT = nc.dram_tensor("xT", [P, DPC * T], fp8, kind="ExternalInput")
    # bias[j, q*O + o] = bias_noise[d0+4q+j, o]
    bias = nc.dram_tensor("bias", [4, (DPC // 4) * O], f32, kind="ExternalInput")
    # res[j, q*O + o] = out[d0+4q+j, o]
    res = nc.dram_tensor("res", [4, (DPC // 4) * O], u8, kind="ExternalOutput")

    with TileContext(nc) as tc:
        _emit(tc, res.ap(), wT.ap(), xT.ap(), bias.ap())
    nc.compile()
    return nc


def prepare_inputs(weight_noise, bias_noise, x):
    """Host-side dtype cast + layout transform + sharding. Exact (0/1 -> fp8)."""
    w8 = np.asarray(weight_noise).astype(FP8)           # [D, O, K]
    # wT[d, p, t, o] = w[d, o, t*128+p]
    wT = np.ascontiguousarray(
        w8.reshape(D, O, T, P).transpose(0, 3, 2, 1)
    ).reshape(D, P, T * O)
    x8 = np.asarray(x).astype(FP8)                      # [D, K]
    xTfull = np.ascontiguousarray(x8.reshape(D, T, P).transpose(2, 0, 1))  # [P, D, T]
    b32 = np.asarray(bias_noise).astype(np.float32)

    in_maps = []
    for c in range(NCORES):
        sl = slice(c * DPC, (c + 1) * DPC)
        # [d, p, t, o] -> [q, p, t, j, o] -> one region per quad (t-major)
        wc = (
            wT[sl]
            .reshape(DPC // 4, 4, P, T, O)
            .transpose(0, 2, 3, 1, 4)
            .reshape(DPC // 4, P, T * 4 * O)
        )
        # bias[j, q*O+o] = bias_noise[d0 + 4q + j, o]
        bc = (
            b32[sl]
            .reshape(DPC // 4, 4, O)
            .transpose(1, 0, 2)
            .reshape(4, (DPC // 4) * O)
        )
        in_maps.append(
            {
                "wT": np.ascontiguousarray(wc),
                "xT": np.ascontiguousarray(xTfull[:, sl, :]).reshape(P, DPC * T),
                "bias": np.ascontiguousarray(bc),
            }
        )
    return in_maps


def run(weight_noise, bias_noise, x, trace=False, **spmd_kwargs):
    """Run on the 8 NeuronCores; returns (bool [D, O] output, BassKernelResults)."""
    from concourse.bass_utils import run_bass_kernel_spmd

    in_maps = prepare_inputs(weight_noise, bias_noise, x)
    if "nc" in _nc_cache:
        nc = _nc_cache["nc"]
    else:
        nc = _nc_cache["nc"] = _build()
    r = run_bass_kernel_spmd(
        nc, in_maps, core_ids=list(range(NCORES)), trace=trace, **spmd_kwargs
    )
    out = np.concatenate(
        [
            r.results[c]["res"]
            .reshape(4, DPC // 4, O)
            .transpose(1, 0, 2)
            .reshape(DPC, O)
            for c in range(NCORES)
        ],
        axis=0,
    )
    return out.astype(bool), r


def kernel(weight_noise, bias_noise, x):
    out, _ = run(weight_noise, bias_noise, x)
    return out
